# revision 1
# baseline (speedup 1.0000x reference)
"""Trainium2 Bass kernel for nn_Block_34256659153605 (dual-branch linear-attention
transformer block). Data-parallel over batch B=8 across 8 NeuronCores; each core
runs the full block for one batch item.

Device layout (per core):
  - Activations live CT ("channels-on-partitions"): X^T tiles [i][c] of shape
    (128, 512) = X^T[128i:128(i+1), 512c:512(c+1)].
  - kp/vp are NT (tokens-on-partitions) tiles (128 tok, 512 ch), so the
    token-softmax denominator is a PE ones-matmul column-sum and ctx = k^T v
    contracts tokens on the partition axis.
  - Matmul operands bf16 (fp32 PSUM accumulation); residual adds are folded
    into PSUM via identity-matmuls (float32r for the fp32 x/y inputs).
  - pos embeddings folded on host: (x+pos)@W = x@W + (pos@W), the latter
    precomputed in numpy and accumulated on-device via identity-matmul.
  - LayerNorm over channels: PE ones-matmul sums -> per-token stats rows ->
    GPSIMD partition_broadcast -> fused DVE/GPSIMD apply.
"""

import os
import sys
import numpy as np

if "/opt/trn_rl_repo" not in sys.path:
    sys.path.insert(0, "/opt/trn_rl_repo")

import ml_dtypes
from contextlib import ExitStack

import concourse.bass as bass
import concourse.mybir as mybir
import concourse.tile as tile
from concourse import bacc
from concourse.masks import make_identity

P = 128
C = 512
H = 4
HID = 4 * C
CT = C // P          # 4 channel blocks
HT = HID // P        # 16 hidden blocks
FD = 512             # token chunk size
EPS = 1e-5

bf16 = mybir.dt.bfloat16
f32 = mybir.dt.float32
f32r = mybir.dt.float32r
AF = mybir.ActivationFunctionType
ALU = mybir.AluOpType

ATTN_W = ["sa_q", "sa_k", "sa_v", "sa_r", "ca_q", "ca_k", "ca_v", "ca_r"]
BIAS_N = ["sa_q", "sa_k", "sa_v", "sa_r", "ca_q", "ca_k", "ca_v", "ca_r",
          "mlp1", "mlp2"]


def build_nc(N=2048, ln_affine=False, biases=frozenset()):
    NCH = N // FD
    nc = bacc.Bacc("TRN2", debug=False)

    dr = {}
    def din(name, shape, dt, kind="ExternalInput"):
        dr[name] = nc.dram_tensor(name, shape, dt, kind=kind).ap()

    din("xT_bf", (C, N), bf16)
    din("yT_bf", (C, N), bf16)
    din("qT_bf", (C, N), bf16)
    for w in ATTN_W:
        din(w + "_w", (C, C), bf16)
    din("mlp_w1", (C, HID), bf16)
    din("mlp_w2", (HID, C), bf16)
    for nm in ["pq_sa_x", "pq_ca_x", "pq_sa_y", "pq_ca_y"]:
        din(nm, (C, N), bf16)
    for nm in ["pk_sa_x", "pk_ca_x", "pk_sa_y", "pk_ca_y"]:
        din(nm, (N, C), bf16)
    for bn in biases:
        din("b_" + bn, (1, HID if bn == "mlp1" else C), bf16)
    if ln_affine:
        din("ln_g", (C,), f32)
        din("ln_b", (C,), f32)
    for nm in ["z_osa", "z_oca", "z_oo", "z_ysa", "z_yca"]:
        din(nm, (C, N), bf16, kind="Internal")
    for nm in ["xsa", "ysa", "xca", "yca", "xml", "yml"]:
        din("st_" + nm, (4, N), f32, kind="Internal")
    for nm in ["xsa", "ysa", "xca", "yca"]:
        din("rq_" + nm, (4, N), bf16, kind="Internal")
    out_d = nc.dram_tensor("yOT", (C, N), f32, kind="ExternalOutput").ap()

    with tile.TileContext(nc) as tc, ExitStack() as ctx:
        consts = ctx.enter_context(tc.tile_pool(name="consts", bufs=1))
        a16 = ctx.enter_context(tc.tile_pool(name="a16", bufs=2))
        a32 = ctx.enter_context(tc.tile_pool(name="a32", bufs=2))
        pmm = ctx.enter_context(tc.tile_pool(name="pmm", bufs=6, space="PSUM"))
        psm = ctx.enter_context(tc.tile_pool(name="psm", bufs=2, space="PSUM"))

        # ---------------- persistent constants ----------------
        def wload(name, dram, nblk, fd):
            t = consts.tile([P, nblk, fd], bf16, name=name)
            nc.sync.dma_start(out=t, in_=dram.rearrange("(i p) c -> p i c", p=P))
            return t

        wsb = {w: wload("w_" + w, dr[w + "_w"], CT, C) for w in ATTN_W}
        w1sb = wload("w_mlp1", dr["mlp_w1"], CT, HID)
        w2sb = wload("w_mlp2", dr["mlp_w2"], HT, C)

        id_bf = consts.tile([P, P], bf16, name="id_bf")
        make_identity(nc, id_bf)
        ones_bf = consts.tile([P, 1], bf16, name="ones_bf")
        nc.vector.memset(ones_bf, 1.0)
        ones_f = consts.tile([P, 1], f32, name="ones_f")
        nc.vector.memset(ones_f, 1.0)
        ones_row = consts.tile([1, FD], bf16, name="ones_row")
        nc.vector.memset(ones_row, 1.0)
        eps_t = consts.tile([P, 1], f32, name="eps_t")
        nc.vector.memset(eps_t, EPS)
        brow = {}
        for bn in biases:
            bt = consts.tile([1, HID if bn == "mlp1" else C], bf16, name="br_" + bn)
            nc.sync.dma_start(out=bt, in_=dr["b_" + bn])
            brow[bn] = bt
        if ln_affine:
            g_col = consts.tile([P, CT], f32, name="g_col")
            b_col = consts.tile([P, CT], f32, name="b_col")
            nc.sync.dma_start(out=g_col, in_=dr["ln_g"].rearrange("(i p) -> p i", p=P))
            nc.sync.dma_start(out=b_col, in_=dr["ln_b"].rearrange("(i p) -> p i", p=P))

        ct_view = lambda d: d.rearrange("(i p) n -> i p n", p=P)
        nt_view = lambda d: d.rearrange("(t p) c -> t p c", p=P)

        def load_ct_chunk(d, c, name, dt=bf16):
            v = ct_view(d)
            pool, tg = (a16, "ld16") if dt == bf16 else (a32, "ld32")
            out = []
            for i in range(CT):
                tl = pool.tile([P, FD], dt, name=name, tag=tg, bufs=6)
                nc.sync.dma_start(out=tl, in_=v[i, :, c * FD:(c + 1) * FD])
                out.append(tl)
            return out

        def bias_ct(ps, bn, blk):
            """psum (out-block blk, tok) += bias[128*blk:...] x ones_row"""
            nc.tensor.matmul(ps, lhsT=brow[bn][0:1, blk * P:(blk + 1) * P],
                             rhs=ones_row, start=False, stop=True)

        def bias_nt(ps, bn):
            """psum (tok, cout) += ones x bias_row"""
            nc.tensor.matmul(ps, lhsT=ones_row[0:1, 0:P], rhs=brow[bn],
                             start=False, stop=True)

        # ---------------- layernorm (over channels) ----------------
        # Incremental: stats matmuls are emitted per chunk right after the
        # chunk's h tiles are produced; rows+apply emitted per 2-chunk batch
        # so h tiles free early (bounds the h32 pool, avoids slot deadlock).
        def dram_bcast_row(a):
            """DRAM AP (1, F) -> broadcast AP (128, F)."""
            return bass.AP(tensor=a.tensor, offset=a.offset,
                           ap=[[0, P]] + [list(d) for d in a.ap[1:]])

        class LNState:
            def __init__(self, tag, zout_dr, final_f32):
                self.tag = tag
                self.zout = zout_dr
                self.final = final_f32
                self.s_ps = psm.tile([P, FD], f32, name=tag + "_sps", tag="sm")
                self.q_ps = psm.tile([P, FD], f32, name=tag + "_qps", tag="sm")
                self.rows = a32.tile([P, 2 * FD], f32, name=tag + "_rows",
                                     tag="rows32", bufs=2)
                self.stdr = dr["st_" + tag]
                self.hf = {}

        def ln_chunk(st, hf_c, hsq_c, c, hb_c):
            """Emit LN stats for chunk c (hf_c/hsq_c: lists over i); on batch
            boundaries do row math (via DRAM re-pack) + bcast + apply + DMA."""
            tag = st.tag
            for i in range(CT):
                nc.tensor.matmul(st.s_ps[32 * c:32 * c + 1, :],
                                 lhsT=ones_bf, rhs=hb_c[i],
                                 start=(i == 0), stop=(i == CT - 1),
                                 tile_position=(0, 32 * c))
            for i in range(CT):
                nc.tensor.matmul(st.q_ps[32 * c:32 * c + 1, :], lhsT=ones_bf,
                                 rhs=hsq_c[i], start=(i == 0),
                                 stop=(i == CT - 1), tile_position=(0, 32 * c))
            # evict this chunk's stat rows and stage them to DRAM (rows live
            # at partition 32c; single-partition APs are legal everywhere)
            r_ = st.rows
            nc.vector.tensor_copy(out=r_[32 * c:32 * c + 1, 0:FD],
                                  in_=st.s_ps[32 * c:32 * c + 1, :])
            nc.vector.tensor_copy(out=r_[32 * c:32 * c + 1, FD:2 * FD],
                                  in_=st.q_ps[32 * c:32 * c + 1, :])
            nc.sync.dma_start(out=st.stdr[0, c * FD:(c + 1) * FD],
                              in_=r_[32 * c:32 * c + 1, 0:FD])
            nc.sync.dma_start(out=st.stdr[1, c * FD:(c + 1) * FD],
                              in_=r_[32 * c:32 * c + 1, FD:2 * FD])
            st.hf[c] = hf_c
            if c % 2 == 0 and c + 1 < NCH:
                return
            c0 = c - 1 if c % 2 == 1 else c
            nb = (c - c0 + 1) * (FD // P)       # packed cols for this batch
            j0 = c0 * (FD // P)
            # re-pack batch rows (tok-major) into (128, nb) via DRAM
            pk = a32.tile([P, 8, 3], f32, name=tag + "_pk", tag="snt", bufs=2)
            pv = lambda row: st.stdr[row, c0 * FD:c0 * FD + nb * P].rearrange(
                "(j p) -> p j", p=P)
            nc.sync.dma_start(out=pk[:, 0:nb, 0], in_=pv(0))
            nc.sync.dma_start(out=pk[:, 0:nb, 1], in_=pv(1))
            m_, q_, t_ = pk[:, 0:nb, 0], pk[:, 0:nb, 1], pk[:, 0:nb, 2]
            nc.vector.tensor_scalar_mul(out=m_, in0=m_, scalar1=1.0 / C)
            nc.vector.tensor_scalar_mul(out=q_, in0=q_, scalar1=1.0 / C)
            nc.vector.tensor_mul(out=t_, in0=m_, in1=m_)
            nc.vector.tensor_sub(out=t_, in0=q_, in1=t_)
            nc.scalar.activation(out=t_, in_=t_, func=AF.Sqrt,
                                 bias=eps_t[:, 0:1], scale=1.0)
            nc.vector.reciprocal(out=t_, in_=t_)           # rstd
            nc.vector.tensor_mul(out=m_, in0=m_, in1=t_)   # m*rstd
            nc.sync.dma_start(out=st.stdr[2, c0 * FD:c0 * FD + nb * P]
                              .rearrange("(j p) -> p j", p=P), in_=t_)
            nc.sync.dma_start(out=st.stdr[3, c0 * FD:c0 * FD + nb * P]
                              .rearrange("(j p) -> p j", p=P), in_=m_)
            zv = ct_view(st.zout)
            for cc in range(c0, c + 1):
                rb = a32.tile([P, FD], f32, name=tag + "_rstdb", tag="bc32", bufs=4)
                mb = a32.tile([P, FD], f32, name=tag + "_mb", tag="bc32", bufs=4)
                nc.sync.dma_start(out=rb, in_=dram_bcast_row(
                    st.stdr[2:3, cc * FD:(cc + 1) * FD]))
                nc.sync.dma_start(out=mb, in_=dram_bcast_row(
                    st.stdr[3:4, cc * FD:(cc + 1) * FD]))
                hb = st.hf.pop(cc)
                for i in range(CT):
                    nc.vector.tensor_mul(out=hb[i], in0=hb[i], in1=rb)
                    if st.final and not ln_affine:
                        zt = a32.tile([P, FD], f32, name=tag + "_zf",
                                      tag="zf32", bufs=3)
                    else:
                        zt = a16.tile([P, FD], bf16, name=tag + "_z",
                                      tag="z16", bufs=5)
                    nc.gpsimd.tensor_sub(out=zt, in0=hb[i], in1=mb)
                    if ln_affine:
                        if st.final:
                            z2 = a32.tile([P, FD], f32, name=tag + "_z2",
                                          tag="zf32", bufs=3)
                        else:
                            z2 = a16.tile([P, FD], bf16, name=tag + "_z2",
                                          tag="z16", bufs=5)
                        nc.vector.tensor_scalar(out=z2, in0=zt,
                                                scalar1=g_col[:, i:i + 1],
                                                scalar2=b_col[:, i:i + 1],
                                                op0=ALU.mult, op1=ALU.add)
                        zt = z2
                    nc.sync.dma_start(out=zv[i, :, cc * FD:(cc + 1) * FD], in_=zt)

        def evict_h(tag, ps):
            h_ = a32.tile([P, FD], f32, name=tag + "_h", tag="h32", bufs=12)
            nc.vector.tensor_copy(out=h_, in_=ps)
            sq = a16.tile([P, FD], bf16, name=tag + "_hsq", tag="sq16", bufs=6)
            nc.scalar.activation(out=sq, in_=ps, func=AF.Square)
            hb = a16.tile([P, FD], bf16, name=tag + "_hb", tag="hb16", bufs=6)
            nc.gpsimd.tensor_copy(out=hb, in_=h_)
            return h_, sq, hb

        def resid_mm(ps, rt, resid_f32, stop):
            nc.tensor.matmul(ps, lhsT=id_bf, rhs=rt, start=False, stop=stop)

        # ---------------- efficient attention ----------------
        def eattn(tag, qin, kin, W, posq, posk, resid, resid_f32, zout,
                  final_f32=False, bq=None, bk=None, bv=None, br=None):
            wq, wk, wv, wr = (wsb[W + "_q"], wsb[W + "_k"],
                              wsb[W + "_v"], wsb[W + "_r"])
            # ---- phase 1: kp / vp / s_k / ctx, token-tile ordered ----
            ctx_ps = [pmm.tile([P, P], f32, name=tag + "_ctx%d" % h, tag="mm")
                      for h in range(H)]
            sk_ps = psm.tile([P, FD], f32, name=tag + "_sk", tag="sm")
            pkv = nt_view(dr[posk]) if posk else None
            for c in range(NCH):
                kint = load_ct_chunk(dr[kin], c, tag + "_kin")
                for tt in range(4):
                    t = 4 * c + tt
                    kp = pmm.tile([P, FD], f32, name=tag + "_kp", tag="mm")
                    for i in range(CT):
                        nc.tensor.matmul(kp, lhsT=kint[i][:, tt * P:(tt + 1) * P],
                                         rhs=wk[:, i, :], start=(i == 0), stop=False)
                    if posk:
                        pk = a16.tile([P, FD], bf16, name=tag + "_pk", tag="pk16", bufs=3)
                        nc.sync.dma_start(out=pk, in_=pkv[t])
                        nc.tensor.matmul(kp, lhsT=id_bf, rhs=pk, start=False,
                                         stop=(bk is None))
                    if bk is not None:
                        bias_nt(kp, bk)
                    ek = a16.tile([P, FD], bf16, name=tag + "_ek", tag="kv16", bufs=5)
                    nc.scalar.activation(out=ek, in_=kp, func=AF.Exp)
                    vp = pmm.tile([P, FD], f32, name=tag + "_vp", tag="mm")
                    for i in range(CT):
                        nc.tensor.matmul(vp, lhsT=kint[i][:, tt * P:(tt + 1) * P],
                                         rhs=wv[:, i, :], start=(i == 0),
                                         stop=(i == CT - 1 and bv is None))
                    if bv is not None:
                        bias_nt(vp, bv)
                    vt = a16.tile([P, FD], bf16, name=tag + "_vt", tag="kv16", bufs=5)
                    nc.vector.tensor_copy(out=vt, in_=vp)
                    nc.tensor.matmul(sk_ps[0:1, :], lhsT=ones_bf, rhs=ek,
                                     start=(t == 0), stop=(t == 4 * NCH - 1))
                    for h in range(H):
                        nc.tensor.matmul(ctx_ps[h],
                                         lhsT=ek[:, h * P:(h + 1) * P],
                                         rhs=vt[:, h * P:(h + 1) * P],
                                         start=(t == 0), stop=(t == 4 * NCH - 1))
            # ---- phase 2: 1/s_k as columns; normalize ctx rows ----
            skrow = a32.tile([P, FD], f32, name=tag + "_skrow", tag="skrow", bufs=2)
            nc.vector.tensor_copy(out=skrow[0:1, :], in_=sk_ps[0:1, :])
            ktp = pmm.tile([P, H], f32, name=tag + "_ktp", tag="mm")
            for h in range(H):
                nc.tensor.transpose(ktp[:, h:h + 1], skrow[0:1, h * P:(h + 1) * P],
                                    ones_f[0:1, 0:1])
            rk = a32.tile([P, H], f32, name=tag + "_rk", tag="rk", bufs=2)
            nc.vector.reciprocal(out=rk, in_=ktp)
            ctx_bf = []
            for h in range(H):
                cb = a16.tile([P, P], bf16, name=tag + "_cbf", tag="cbf", bufs=8)
                nc.vector.tensor_scalar_mul(out=cb, in0=ctx_ps[h],
                                            scalar1=rk[:, h:h + 1])
                ctx_bf.append(cb)
            # ---- phase 3: qp / q-softmax / att / reproj+resid, chunk ordered ----
            pqv = ct_view(dr[posq]) if posq else None
            rv = ct_view(resid)
            lst = LNState(tag, zout, final_f32)
            for c in range(NCH):
                qint = load_ct_chunk(dr[qin], c, tag + "_qin")
                sq_ps = pmm.tile([P, FD], f32, name=tag + "_sq", tag="mm")
                eq = []
                for m in range(CT):
                    ps = pmm.tile([P, FD], f32, name=tag + "_qp", tag="mm")
                    for i in range(CT):
                        nc.tensor.matmul(ps, lhsT=wq[:, i, m * P:(m + 1) * P],
                                         rhs=qint[i], start=(i == 0),
                                         stop=(i == CT - 1 and posq is None
                                               and bq is None))
                    if posq:
                        pq = a16.tile([P, FD], bf16, name=tag + "_pq", tag="pk16", bufs=3)
                        nc.sync.dma_start(out=pq, in_=pqv[m, :, c * FD:(c + 1) * FD])
                        nc.tensor.matmul(ps, lhsT=id_bf, rhs=pq, start=False,
                                         stop=(bq is None))
                    if bq is not None:
                        bias_ct(ps, bq, m)
                    e = a16.tile([P, FD], bf16, name=tag + "_eq", tag="eq16", bufs=6)
                    nc.scalar.activation(out=e, in_=ps, func=AF.Exp)
                    eq.append(e)
                    nc.tensor.matmul(sq_ps[32 * m:32 * m + 1, :], lhsT=ones_bf,
                                     rhs=e, start=True, stop=True,
                                     tile_position=(0, 32 * m))
                rqr = a16.tile([P, FD], bf16, name=tag + "_rqr", tag="rqr", bufs=2)
                with nc.allow_low_precision(reason="bf16 softmax recip ok"):
                    for h in range(H):
                        nc.vector.reciprocal(out=rqr[32 * h:32 * h + 1, :],
                                             in_=sq_ps[32 * h:32 * h + 1, :])
                rq_dr = dr["rq_" + tag]
                for h in range(H):
                    nc.sync.dma_start(out=rq_dr[h, c * FD:(c + 1) * FD],
                                      in_=rqr[32 * h:32 * h + 1, :])
                for h in range(H):
                    rqb = a16.tile([P, FD], bf16, name=tag + "_rqb", tag="bc16", bufs=6)
                    nc.sync.dma_start(out=rqb, in_=dram_bcast_row(
                        rq_dr[h:h + 1, c * FD:(c + 1) * FD]))
                    aps = pmm.tile([P, FD], f32, name=tag + "_aps", tag="mm")
                    nc.tensor.matmul(aps, lhsT=ctx_bf[h], rhs=eq[h],
                                     start=True, stop=True)
                    ab = a16.tile([P, FD], bf16, name=tag + "_ab", tag="att16", bufs=6)
                    nc.vector.tensor_mul(out=ab, in0=aps, in1=rqb)
                    eq[h] = None
                    eq.append(ab)  # keep refs ordered: att tiles at eq[H+h]
                att = eq[H:]
                rts = []
                for i in range(CT):
                    rt = a16.tile([P, FD], bf16, name=tag + "_rt", tag="ld16", bufs=6)
                    nc.sync.dma_start(out=rt, in_=rv[i, :, c * FD:(c + 1) * FD])
                    rts.append(rt)
                hfc, hsqc, hbc = [], [], []
                for i in range(CT):
                    ps = pmm.tile([P, FD], f32, name=tag + "_rp", tag="mm")
                    for hh in range(CT):
                        nc.tensor.matmul(ps, lhsT=wr[:, hh, i * P:(i + 1) * P],
                                         rhs=att[hh], start=(hh == 0), stop=False)
                    resid_mm(ps, rts[i], resid_f32, stop=(br is None))
                    if br is not None:
                        bias_ct(ps, br, i)
                    h_, sq_, hb_ = evict_h(tag, ps)
                    hfc.append(h_)
                    hsqc.append(sq_)
                    hbc.append(hb_)
                ln_chunk(lst, hfc, hsqc, c, hbc)

        # ---------------- MLP ----------------
        def mlp(tag, zin, zout, final_f32, b1=None, b2=None):
            rv = ct_view(dr[zin])
            lst = LNState(tag, zout, final_f32)
            for c in range(NCH):
                zint = load_ct_chunk(dr[zin], c, tag + "_zin")
                u = []
                for ht in range(HT):
                    ps = pmm.tile([P, FD], f32, name=tag + "_f1", tag="mm")
                    for i in range(CT):
                        nc.tensor.matmul(ps, lhsT=w1sb[:, i, ht * P:(ht + 1) * P],
                                         rhs=zint[i], start=(i == 0),
                                         stop=(i == CT - 1 and b1 is None))
                    if b1 is not None:
                        bias_ct(ps, b1, ht)
                    ut = a16.tile([P, FD], bf16, name=tag + "_u", tag="u16", bufs=20)
                    if ht % 2 == 0:
                        nc.scalar.activation(out=ut, in_=ps, func=AF.Relu)
                    else:
                        nc.vector.tensor_scalar_max(out=ut, in0=ps, scalar1=0.0)
                    u.append(ut)
                rts = []
                for i in range(CT):
                    rt = a16.tile([P, FD], bf16, name=tag + "_rt", tag="ld16", bufs=6)
                    nc.sync.dma_start(out=rt, in_=rv[i, :, c * FD:(c + 1) * FD])
                    rts.append(rt)
                hfc, hsqc, hbc = [], [], []
                for i in range(CT):
                    ps = pmm.tile([P, FD], f32, name=tag + "_f2", tag="mm")
                    for ht in range(HT):
                        nc.tensor.matmul(ps, lhsT=w2sb[:, ht, i * P:(i + 1) * P],
                                         rhs=u[ht], start=(ht == 0), stop=False)
                    resid_mm(ps, rts[i], False, stop=(b2 is None))
                    if b2 is not None:
                        bias_ct(ps, b2, i)
                    h_, sq_, hb_ = evict_h(tag, ps)
                    hfc.append(h_)
                    hsqc.append(sq_)
                    hbc.append(hb_)
                ln_chunk(lst, hfc, hsqc, c, hbc)

        bb = lambda n: (n if n in biases else None)

        eattn("xsa", "xT_bf", "xT_bf", "sa", "pq_sa_x", "pk_sa_x",
              dr["xT_bf"], False, dr["z_osa"],
              bq=bb("sa_q"), bk=bb("sa_k"), bv=bb("sa_v"), br=bb("sa_r"))
        eattn("ysa", "yT_bf", "yT_bf", "sa", "pq_sa_y", "pk_sa_y",
              dr["yT_bf"], False, dr["z_ysa"],
              bq=bb("sa_q"), bk=bb("sa_k"), bv=bb("sa_v"), br=bb("sa_r"))
        eattn("xca", "qT_bf", "z_osa", "ca", "pq_ca_x", "pk_ca_x",
              dr["z_osa"], False, dr["z_oca"],
              bq=bb("ca_q"), bk=bb("ca_k"), bv=bb("ca_v"), br=bb("ca_r"))
        mlp("xml", "z_oca", dr["z_oo"], False, b1=bb("mlp1"), b2=bb("mlp2"))
        eattn("yca", "z_oo", "z_ysa", "ca", "pq_ca_y", "pk_ca_y",
              dr["z_ysa"], False, dr["z_yca"],
              bq=bb("ca_q"), bk=bb("ca_k"), bv=bb("ca_v"), br=bb("ca_r"))
        mlp("yml", "z_yca", out_d, True, b1=bb("mlp1"), b2=bb("mlp2"))

    nc.compile()
    return nc


# ======================= host side =======================

_NC_CACHE = {}
LAST_RESULT = None


def _get_nc(N, ln_affine, biases):
    key = (N, ln_affine, tuple(sorted(biases)))
    if key not in _NC_CACHE:
        _NC_CACHE[key] = build_nc(N, ln_affine, frozenset(biases))
    return _NC_CACHE[key]


def _bf(a):
    return np.ascontiguousarray(a.astype(ml_dtypes.bfloat16))


def host_prep(inputs, N):
    """Common (core-independent) in_map entries."""
    ws = {w: np.asarray(inputs[w + "_w"], np.float32) for w in ATTN_W}
    posx = np.asarray(inputs["pos_x"], np.float32)[0]  # (N, C)
    posy = np.asarray(inputs["pos_y"], np.float32)[0]
    m = {}
    for w in ATTN_W:
        m[w + "_w"] = _bf(ws[w])
    m["mlp_w1"] = _bf(np.asarray(inputs["mlp_w1"], np.float32))
    m["mlp_w2"] = _bf(np.asarray(inputs["mlp_w2"], np.float32))
    m["pq_sa_x"] = _bf((posx @ ws["sa_q"]).T)
    m["pq_ca_x"] = _bf((posx @ ws["ca_q"]).T)
    m["pq_sa_y"] = _bf((posy @ ws["sa_q"]).T)
    m["pq_ca_y"] = _bf((posy @ ws["ca_q"]).T)
    m["pk_sa_x"] = _bf(posx @ ws["sa_k"])
    m["pk_ca_x"] = _bf(posx @ ws["ca_k"])
    m["pk_sa_y"] = _bf(posy @ ws["sa_k"])
    m["pk_ca_y"] = _bf(posy @ ws["ca_k"])
    bias_arr = {"sa_q": "sa_q_b", "sa_k": "sa_k_b", "sa_v": "sa_v_b",
                "sa_r": "sa_r_b", "ca_q": "ca_q_b", "ca_k": "ca_k_b",
                "ca_v": "ca_v_b", "ca_r": "ca_r_b",
                "mlp1": "mlp_b1", "mlp2": "mlp_b2"}
    biases = set()
    for bn, an in bias_arr.items():
        arr = np.asarray(inputs[an], np.float32)
        if np.any(arr != 0):
            biases.add(bn)
            m["b_" + bn] = _bf(arr.reshape(1, -1))
    g = np.asarray(inputs["ln_g"], np.float32)
    b = np.asarray(inputs["ln_b"], np.float32)
    ln_affine = bool(np.any(g != 1) or np.any(b != 0))
    if ln_affine:
        m["ln_g"] = np.ascontiguousarray(g)
        m["ln_b"] = np.ascontiguousarray(b)
    return m, biases, ln_affine


def core_inputs(inputs, b):
    x = np.asarray(inputs["x"], np.float32)[b]
    y = np.asarray(inputs["y"], np.float32)[b]
    q = np.asarray(inputs["q"], np.float32)[b]
    return {"xT_bf": _bf(x.T), "yT_bf": _bf(y.T), "qT_bf": _bf(q.T)}


def kernel(**inputs):
    from concourse import bass_utils
    N = np.asarray(inputs["x"]).shape[1]
    B = np.asarray(inputs["x"]).shape[0]
    common, biases, ln_affine = host_prep(inputs, N)
    nc = _get_nc(N, ln_affine, biases)
    in_maps = []
    for b in range(B):
        m = dict(common)
        m.update(core_inputs(inputs, b))
        in_maps.append(m)
    res = bass_utils.run_bass_kernel_spmd(nc, in_maps, core_ids=list(range(B)))
    global LAST_RESULT
    LAST_RESULT = res
    out = np.stack([r["yOT"].T for r in res.results], axis=0)
    return np.ascontiguousarray(out.astype(np.float32))

